# revision 8
# baseline (speedup 1.0000x reference)
"""Trainium2 Bass/Tile kernel: batched dot-product attention with length masking.

Problem: queries/keys/values [32, 1024, 128] f32, valid_length [32] int64.
  out = softmax(mask(Q K^T / sqrt(128))) @ V

Strategy (v2 — balanced k-block packing, host-side denominator):
  - Work unit = one 128-wide k-block of one batch. Total blocks
    N = sum_b ceil(vl_b/128) (=136 for the reference input). Attention is
    associative over k (partial numerator + partial sum-of-exp combine on
    host), so blocks of one batch can be split across cores at will.
  - SPMD program = G slots with fixed block counts sizes[g]
    (sum = ceil(N/8) = 17). A DP packer cuts batches into contiguous
    k-range pieces so every core's slot g holds exactly sizes[g] blocks
    (padding pieces are all-zero and harmless). Per-core work is 17 blocks
    vs 20 for the per-slot-max batch scheme.
  - Masking costs nothing on device: host zeroes masked K columns and V
    rows. Then S=0 and exp(0)=1 on masked columns; the numerator is clean
    (V rows are 0) and the denominator over-counts by exactly the masked
    column count, which the host subtracts. No bias tensors, uniform exps.
  - Device per piece (q [128,Q] fp16 = Q^T, k [128,s*128] fp16 = K^T,
    v [128, s*128] fp16 V partition-major per block):
      S^T[k,q] = K_blk.T @ Q^T          PE (512-row moving passes)
      P^T_blk  = exp(S^T*scale)         ScalarE PSUM->SBUF fp16
      pacc     = sum_blk P^T_blk        DVE adds (s-1 per piece)
      O^T[v,q] = sum_blk V_blk @ P^T    PE, V stationary, PSUM accumulate
    Outputs per piece: oT [128,Q] fp16 and pacc [128,Q] fp16 (P itself for
    1-block pieces). Host: den = pacc.sum(partitions) - n_masked;
    out = sum_pieces oT^T / den. No den matmul on the PE and no PSUM bank
    for it -> s_pool gets 3 PSUM buffers (deeper S pipelining).
  - ScalarE runs ONLY the exp stream (the critical path: 17 x ~1.04us);
    all steady-state DMA issues ride the sync ring. Head: k0 on sync,
    q0 halves on scalar+vector in parallel so the first exp fires ~4.5us
    after kernel entry. A dummy 1-column exp hoists the ~1.3us
    ACT_TABLE_LOAD into the DMA shadow; dummy matmuls ramp the PE p-state.
  - Slot order: smallest piece first (tiny k0 -> earliest exp stream
    start), then descending; a small slot last keeps the tail short.
"""

import numpy as np

import concourse.tile as tile
from concourse import bacc, mybir
from concourse.bass_utils import run_bass_kernel_spmd

B, Q, K, D = 32, 1024, 1024, 128
N_CORES = 8
QH = 512
SCALE = float(1.0 / np.sqrt(D))
N_WARM = 8

LAST_RESULTS = None
_NC_CACHE: dict = {}


# ---------------------------------------------------------------- packing

def _compositions(n, parts):
    """All count-tuples over `parts` (descending) summing to n."""
    out = []

    def rec(i, left, cur):
        if left == 0:
            out.append(tuple(cur + [0] * (len(parts) - len(cur))))
            return
        if i == len(parts):
            return
        for c in range(left // parts[i], -1, -1):
            rec(i + 1, left - c * parts[i], cur + [c])

    rec(0, n, [])
    return out


def _solve_sizes(kbs, sizes, n_cores):
    """Cut batches (block counts kbs) into pieces matching the cell multiset
    {8 x s for s in sizes}. Returns per-batch piece-size lists or None."""
    usizes = sorted(set(sizes), reverse=True)
    cap = tuple(sizes.count(s) * n_cores for s in usizes)
    comps = {kb: _compositions(kb, usizes) for kb in set(kbs)}
    if any(not v for v in comps.values()):
        return None

    from functools import lru_cache

    kbs_t = tuple(kbs)

    @lru_cache(maxsize=None)
    def rec(i, remaining):
        if i == len(kbs_t):
            return ()
        for comp in comps[kbs_t[i]]:
            if any(c > r for c, r in zip(comp, remaining)):
                continue
            sub = rec(i + 1, tuple(r - c for r, c in zip(remaining, comp)))
            if sub is not None:
                return (comp,) + sub
        return None

    res = rec(0, cap)
    if res is None:
        return None
    return [
        [s for s, c in zip(usizes, comp) for _ in range(c)] for comp in res
    ]


def _find_packing(vl, n_cores=N_CORES, max_groups=6):
    """Returns (sizes, per-batch piece lists). sizes sums to the per-core
    block budget. Falls back to the per-slot-max whole-batch scheme."""
    kbs = [max(1, int(np.ceil(v / 128.0))) for v in vl]
    n_total = sum(kbs)
    t0 = int(np.ceil(n_total / n_cores))
    for T in range(t0, t0 + 3):
        cands = []

        def gen(left, maxp, cur):
            if left == 0:
                cands.append(list(cur))
                return
            if len(cur) >= max_groups:
                return
            for p in range(min(maxp, left), 0, -1):
                gen(left - p, p, cur + [p])

        gen(T, K // 128, [])
        cands.sort(key=lambda s: (len(s), -min(s)))
        for sizes in cands:
            sol = _solve_sizes(kbs, sizes, n_cores)
            if sol is not None:
                return sizes, sol
    # fallback: sorted whole-batch slots (baseline scheme)
    order = np.argsort(-np.asarray(vl), kind="stable")
    assign = order.reshape(-1, n_cores)
    sizes = [max(1, int(np.ceil(np.asarray(vl)[assign[j]].max() / 128.0)))
             for j in range(assign.shape[0])]
    sol = [[] for _ in kbs]
    for j in range(assign.shape[0]):
        for b in assign[j]:
            sol[b] = [sizes[j]]
    return sizes, sol


def _order_slots(sizes):
    """Program order tuned for the DMA FIFO: a smallish slot first (its k is
    cheap to fetch, so the exp stream starts early), then descending (the big
    slot's exps cover every later load), 1-block slots last (short tail)."""
    s = sorted(sizes, reverse=True)
    # first: the smallest size >= 2 if one exists, else the smallest
    cand = [x for x in s if x >= 2]
    first = min(cand) if cand else s[-1]
    s.remove(first)
    return [first] + s


# ---------------------------------------------------------------- device

def _body(tc, qs, ks, vs, oTs, paccs, sizes):
    nc = tc.nc
    f32 = mybir.dt.float32
    f16 = mybir.dt.float16
    AF = mybir.ActivationFunctionType
    G = len(sizes)
    smax = max(sizes)

    with (
        tc.tile_pool(name="q", bufs=3) as q_pool,
        tc.tile_pool(name="k", bufs=3) as k_pool,
        tc.tile_pool(name="v", bufs=3) as v_pool,
        tc.tile_pool(name="p", bufs=2) as p_pool,
        tc.tile_pool(name="pa", bufs=2) as pa_pool,
        tc.tile_pool(name="eo", bufs=2) as e_pool,
        tc.tile_pool(name="const", bufs=1) as c_pool,
        tc.tile_pool(name="spsum", bufs=2, space="PSUM") as s_pool,
        tc.tile_pool(name="opsum", bufs=2, space="PSUM") as o_pool,
    ):
        def load_slot(g):
            s = sizes[g]
            sc = s * 128
            q_sb = q_pool.tile([128, Q], f16, tag="q", name=f"q{g}")
            k_sb = k_pool.tile([128, smax * 128], f16, tag="k", name=f"k{g}")
            v_sb = v_pool.tile([128, smax * 128], f16, tag="v", name=f"v{g}")
            if g == 0:
                # transfers drain a single global DMA-engine FIFO; order the
                # head so k0, q0h0, q0h1, k1, q1 land back-to-back and the
                # exp stream starts dense. v0 is only needed at PV(g0) so it
                # rides behind k1/q1.
                nc.sync.dma_start(out=k_sb[:, 0:sc], in_=ks[g][:])
                nc.scalar.dma_start(out=q_sb[:, 0:QH], in_=qs[g][:, 0:QH])
                nc.sync.dma_start(out=q_sb[:, QH:Q], in_=qs[g][:, QH:Q])
                nc.sync.dma_start(out=v_sb[:, 0:sc], in_=vs[g][:])
            elif g == 1:
                # scalar's issue queue is free until the first exp; its k1/q1
                # issues finish DGE earlier than sync's would (sync is still
                # issuing slot-0), landing them right behind the q0 halves
                nc.scalar.dma_start(out=k_sb[:, 0:sc], in_=ks[g][:])
                nc.scalar.dma_start(out=q_sb[:], in_=qs[g][:])
                nc.sync.dma_start(out=v_sb[:, 0:sc], in_=vs[g][:])
            else:
                nc.sync.dma_start(out=k_sb[:, 0:sc], in_=ks[g][:])
                nc.sync.dma_start(out=q_sb[:], in_=qs[g][:])
                nc.sync.dma_start(out=v_sb[:, 0:sc], in_=vs[g][:])
            return q_sb, k_sb, v_sb

        def s_exp_one(g, blk, q_sb, k_sb, p_all):
            s_ps = s_pool.tile([128, Q], f32, tag="s", name=f"s{g}_{blk}")
            lhsT = k_sb[:, blk * 128 : (blk + 1) * 128]
            for qh in range(Q // QH):
                nc.tensor.matmul(
                    s_ps[:, qh * QH : (qh + 1) * QH],
                    lhsT,
                    q_sb[:, qh * QH : (qh + 1) * QH],
                    start=True,
                    stop=True,
                )
            nc.scalar.activation(
                p_all[:, blk * Q : (blk + 1) * Q], s_ps[:], AF.Exp, scale=SCALE
            )

        def s_exp_head(g, q_sb, k_sb):
            p_all = p_pool.tile([128, smax * Q], f16, tag="p", name=f"p{g}")
            for blk in range(min(2, sizes[g])):
                s_exp_one(g, blk, q_sb, k_sb, p_all)
            return p_all

        def s_exp_tail(g, q_sb, k_sb, p_all, start_blk):
            s = sizes[g]
            if s == 1:
                return None
            pacc = pa_pool.tile([128, Q], f16, tag="pa", name=f"pa{g}")
            nc.vector.tensor_add(pacc[:], p_all[:, 0:Q], p_all[:, Q : 2 * Q])
            for blk in range(2, s):
                if blk >= start_blk:
                    s_exp_one(g, blk, q_sb, k_sb, p_all)
                nc.vector.tensor_add(
                    pacc[:], pacc[:], p_all[:, blk * Q : (blk + 1) * Q]
                )
            return pacc

        def pv_out(g, p_all, v_sb, pacc, nxt=None):
            s = sizes[g]
            last = g == G - 1
            # pacc output: P itself for 1-block pieces (no DVE work). The
            # last slot's goes on scalar (its exp stream is over) so it
            # overlaps the sync-side oT traffic.
            pacc_src = pacc[:] if pacc is not None else p_all[:, 0:Q]
            if last:
                nc.scalar.dma_start(out=paccs[g], in_=pacc_src)
            else:
                nc.sync.dma_start(out=paccs[g], in_=pacc_src)
            o_ps = [
                o_pool.tile([128, QH], f32, tag=f"o{qh}", name=f"o{g}_{qh}")
                for qh in range(Q // QH)
            ]
            for blk in range(s):
                for qh in range(Q // QH):
                    nc.tensor.matmul(
                        o_ps[qh][:],
                        v_sb[:, blk * 128 : (blk + 1) * 128],
                        p_all[:, blk * Q + qh * QH : blk * Q + (qh + 1) * QH],
                        start=(blk == 0),
                        stop=(blk == s - 1),
                    )
            # next slot's 3rd S tile + exp ride between PV and the evac so
            # the ScalarE stream stays dense across the slot boundary
            if nxt is not None:
                ng, nq, nk, np_all = nxt
                if sizes[ng] > 2:
                    s_exp_one(ng, 2, nq, nk, np_all)
            o_all = e_pool.tile([128, Q], f16, tag="oall", name=f"oall{g}")
            if last:
                # tail: halves split across DVE and ScalarE (exp stream done),
                # DMAs split across sync and scalar rings
                nc.vector.tensor_copy(o_all[:, 0:QH], o_ps[0][:])
                nc.sync.dma_start(out=oTs[g][:, 0:QH], in_=o_all[:, 0:QH])
                nc.scalar.copy(o_all[:, QH:Q], o_ps[1][:])
                nc.scalar.dma_start(out=oTs[g][:, QH:Q], in_=o_all[:, QH:Q])
            else:
                for qh in range(Q // QH):
                    nc.vector.tensor_copy(
                        o_all[:, qh * QH : (qh + 1) * QH], o_ps[qh][:]
                    )
                nc.sync.dma_start(out=oTs[g], in_=o_all[:])

        # slot-0 loads first so their DMA issues are each ring's first work
        loads = [load_slot(0)]

        # dummy 1-column exp hoists the compiler-inserted ACT_TABLE_LOAD
        # (~1.3us) into the slot-0 DMA shadow
        scratch = c_pool.tile([128, 1], f16, tag="scratch", bufs=1)
        nc.gpsimd.memset(scratch[:], 1.0)
        nc.scalar.activation(scratch[:], scratch[:], AF.Exp, scale=1.0)

        # dummy matmuls ramp the PE p-state while slot-0 loads are in flight
        warm_w = c_pool.tile([128, QH], f16, tag="warmw", bufs=1)
        nc.gpsimd.memset(warm_w[:], 0.0)
        for w in range(N_WARM):
            warm_ps = s_pool.tile([128, QH], f32, tag="s", name=f"warm{w}")
            nc.tensor.matmul(warm_ps[:], warm_w[:, 0:128], warm_w[:],
                             start=True, stop=True)

        p_alls = [s_exp_head(0, loads[0][0], loads[0][1])]
        for g in range(G):
            if g + 1 < G:
                loads.append(load_slot(g + 1))
            q_sb, k_sb, v_sb = loads[g]
            pacc = s_exp_tail(g, q_sb, k_sb, p_alls[g],
                              start_blk=2 if g == 0 else 3)
            nxt = None
            if g + 1 < G:
                lq, lk, lv = loads[g + 1]
                p_alls.append(s_exp_head(g + 1, lq, lk))
                nxt = (g + 1, lq, lk, p_alls[g + 1])
            pv_out(g, p_alls[g], v_sb, pacc, nxt)


def _build(sizes):
    key = tuple(sizes)
    if key in _NC_CACHE:
        return _NC_CACHE[key]
    nc = bacc.Bacc("TRN2", target_bir_lowering=False, debug=False,
                   enable_asserts=False, enable_partition_id=False)
    f16 = mybir.dt.float16
    qs, ks, vs, oTs, paccs = [], [], [], [], []
    for g, s in enumerate(sizes):
        sc = s * 128
        qs.append(nc.dram_tensor(f"q{g}", [D, Q], f16,
                                 kind="ExternalInput").ap())
        ks.append(nc.dram_tensor(f"k{g}", [D, sc], f16,
                                 kind="ExternalInput").ap())
        vs.append(nc.dram_tensor(f"v{g}", [128, sc], f16,
                                 kind="ExternalInput").ap())
        oTs.append(nc.dram_tensor(f"oT{g}", [D, Q], f16,
                                  kind="ExternalOutput").ap())
        paccs.append(nc.dram_tensor(f"pacc{g}", [128, Q], f16,
                                    kind="ExternalOutput").ap())
    with tile.TileContext(nc) as tc:
        _body(tc, qs, ks, vs, oTs, paccs, sizes)
    nc.compile()
    _NC_CACHE[key] = nc
    return nc


# ---------------------------------------------------------------- host

def _prep(queries, keys, values, valid_length):
    """Returns (in_maps, pieces_by_cell, sizes).
    pieces_by_cell[(core, slot)] = (batch, k0_block, n_blocks) or None."""
    vl = np.asarray(valid_length).astype(np.int64).reshape(B)
    sizes_ms, per_batch = _find_packing(vl)
    sizes = _order_slots(sizes_ms)
    G = len(sizes)

    # cut each batch into contiguous pieces (largest piece first at k0=0)
    pieces_by_size: dict[int, list] = {}
    for b in range(B):
        k0 = 0
        for s in sorted(per_batch[b], reverse=True):
            pieces_by_size.setdefault(s, []).append((b, k0, s))
            k0 += s
    # fill cells slot by slot
    cells = {}
    for g in range(G):
        s = sizes[g]
        for c in range(N_CORES):
            lst = pieces_by_size.get(s, [])
            cells[(c, g)] = lst.pop() if lst else None
    assert all(not v for v in pieces_by_size.values()), "unassigned pieces"

    q = np.asarray(queries, dtype=np.float32)
    k = np.asarray(keys, dtype=np.float32)
    v = np.asarray(values, dtype=np.float32)
    qT_all = np.ascontiguousarray(q.transpose(0, 2, 1)).astype(np.float16)
    pos = np.arange(K)

    in_maps = []
    for c in range(N_CORES):
        m = {}
        for g in range(G):
            s = sizes[g]
            sc = s * 128
            piece = cells[(c, g)]
            if piece is None:
                m[f"q{g}"] = np.zeros((D, Q), np.float16)
                m[f"k{g}"] = np.zeros((D, sc), np.float16)
                m[f"v{g}"] = np.zeros((128, sc), np.float16)
                continue
            bi, k0, s_ = piece
            lo, hi = k0 * 128, k0 * 128 + sc
            valid = (pos[lo:hi] < vl[bi])[None, :]
            m[f"q{g}"] = qT_all[bi]
            m[f"k{g}"] = np.where(
                valid, k[bi, lo:hi].T, np.float32(0.0)
            ).astype(np.float16)
            vz = np.where(valid.T, v[bi, lo:hi], np.float32(0.0))
            m[f"v{g}"] = np.ascontiguousarray(
                vz.reshape(s, 128, D).transpose(1, 0, 2).reshape(128, sc)
            ).astype(np.float16)
        in_maps.append(m)
    return in_maps, cells, sizes, vl


def kernel(queries, keys, values, valid_length):
    global LAST_RESULTS
    in_maps, cells, sizes, vl = _prep(queries, keys, values, valid_length)
    nc = _build(sizes)
    res = run_bass_kernel_spmd(nc, in_maps, list(range(N_CORES)))
    LAST_RESULTS = res
    num = np.zeros((B, Q, D), np.float32)
    den = np.zeros((B, Q), np.float32)
    for c in range(N_CORES):
        rc = res.results[c]
        for g in range(len(sizes)):
            piece = cells[(c, g)]
            if piece is None:
                continue
            bi, k0, s = piece
            oT = np.asarray(rc[f"oT{g}"]).astype(np.float32)      # [D, Q]
            pacc = np.asarray(rc[f"pacc{g}"]).astype(np.float32)  # [128, Q]
            n_valid = int(np.clip(vl[bi] - 128 * k0, 0, 128 * s))
            n_masked = 128 * s - n_valid
            num[bi] += oT.T
            den[bi] += pacc.sum(axis=0) - np.float32(n_masked)
    return num / den[:, :, None]


# revision 13
# speedup vs baseline: 1.0852x; 1.0852x over previous
"""Trainium2 Bass/Tile kernel: batched dot-product attention with length masking.

Problem: queries/keys/values [32, 1024, 128] f32, valid_length [32] int64.
  out = softmax(mask(Q K^T / sqrt(128))) @ V

Strategy (v2 — balanced k-block packing, host-side denominator):
  - Work unit = one 128-wide k-block of one batch. Total blocks
    N = sum_b ceil(vl_b/128) (=136 for the reference input). Attention is
    associative over k (partial numerator + partial sum-of-exp combine on
    host), so blocks of one batch can be split across cores at will.
  - SPMD program = G slots with fixed block counts sizes[g]
    (sum = ceil(N/8) = 17). A DP packer cuts batches into contiguous
    k-range pieces so every core's slot g holds exactly sizes[g] blocks
    (padding pieces are all-zero and harmless). Per-core work is 17 blocks
    vs 20 for the per-slot-max batch scheme.
  - Masking costs nothing on device: host zeroes masked K columns and V
    rows. Then S=0 and exp(0)=1 on masked columns; the numerator is clean
    (V rows are 0) and the denominator over-counts by exactly the masked
    column count, which the host subtracts. No bias tensors, uniform exps.
  - Device per piece (q [128,Q] fp16 = Q^T, k [128,s*128] fp16 = K^T,
    v [128, s*128] fp16 V partition-major per block):
      S^T[k,q] = K_blk.T @ Q^T          PE (512-row moving passes)
      P^T_blk  = exp(S^T*scale)         ScalarE PSUM->SBUF fp16
      pacc     = sum_blk P^T_blk        DVE adds (s-1 per piece)
      O^T[v,q] = sum_blk V_blk @ P^T    PE, V stationary, PSUM accumulate
    Outputs per piece: oT [128,Q] fp16 and pacc [128,Q] fp16 (P itself for
    1-block pieces). Host: den = pacc.sum(partitions) - n_masked;
    out = sum_pieces oT^T / den. No den matmul on the PE and no PSUM bank
    for it -> s_pool gets 3 PSUM buffers (deeper S pipelining).
  - ScalarE runs ONLY the exp stream (the critical path: 17 x ~1.04us);
    all steady-state DMA issues ride the sync ring. Head: k0 on sync,
    q0 halves on scalar+vector in parallel so the first exp fires ~4.5us
    after kernel entry. A dummy 1-column exp hoists the ~1.3us
    ACT_TABLE_LOAD into the DMA shadow; dummy matmuls ramp the PE p-state.
  - Slot order: smallest piece first (tiny k0 -> earliest exp stream
    start), then descending; a small slot last keeps the tail short.
"""

import numpy as np

import concourse.tile as tile
from concourse import bacc, mybir
from concourse.bass_utils import run_bass_kernel_spmd

B, Q, K, D = 32, 1024, 1024, 128
N_CORES = 8
QH = 512
SCALE = float(1.0 / np.sqrt(D))
N_WARM = 6

LAST_RESULTS = None
_NC_CACHE: dict = {}


# ---------------------------------------------------------------- packing

def _compositions(n, parts):
    """All count-tuples over `parts` (descending) summing to n."""
    out = []

    def rec(i, left, cur):
        if left == 0:
            out.append(tuple(cur + [0] * (len(parts) - len(cur))))
            return
        if i == len(parts):
            return
        for c in range(left // parts[i], -1, -1):
            rec(i + 1, left - c * parts[i], cur + [c])

    rec(0, n, [])
    return out


def _solve_sizes(kbs, sizes, n_cores):
    """Cut batches (block counts kbs) into pieces matching the cell multiset
    {8 x s for s in sizes}. Returns per-batch piece-size lists or None."""
    usizes = sorted(set(sizes), reverse=True)
    cap = tuple(sizes.count(s) * n_cores for s in usizes)
    comps = {kb: _compositions(kb, usizes) for kb in set(kbs)}
    if any(not v for v in comps.values()):
        return None

    from functools import lru_cache

    kbs_t = tuple(kbs)

    @lru_cache(maxsize=None)
    def rec(i, remaining):
        if i == len(kbs_t):
            return ()
        for comp in comps[kbs_t[i]]:
            if any(c > r for c, r in zip(comp, remaining)):
                continue
            sub = rec(i + 1, tuple(r - c for r, c in zip(remaining, comp)))
            if sub is not None:
                return (comp,) + sub
        return None

    res = rec(0, cap)
    if res is None:
        return None
    return [
        [s for s, c in zip(usizes, comp) for _ in range(c)] for comp in res
    ]


def _find_packing(vl, n_cores=N_CORES, max_groups=6):
    """Returns (sizes, per-batch piece lists). sizes sums to the per-core
    block budget. Falls back to the per-slot-max whole-batch scheme."""
    kbs = [max(1, int(np.ceil(v / 128.0))) for v in vl]
    n_total = sum(kbs)
    t0 = int(np.ceil(n_total / n_cores))
    for T in range(t0, t0 + 3):
        cands = []

        def gen(left, maxp, cur):
            if left == 0:
                cands.append(list(cur))
                return
            if len(cur) >= max_groups:
                return
            for p in range(min(maxp, left), 0, -1):
                gen(left - p, p, cur + [p])

        gen(T, K // 128, [])
        cands.sort(key=lambda s: (len(s), -min(s)))
        for sizes in cands:
            sol = _solve_sizes(kbs, sizes, n_cores)
            if sol is not None:
                return sizes, sol
    # fallback: sorted whole-batch slots (baseline scheme)
    order = np.argsort(-np.asarray(vl), kind="stable")
    assign = order.reshape(-1, n_cores)
    sizes = [max(1, int(np.ceil(np.asarray(vl)[assign[j]].max() / 128.0)))
             for j in range(assign.shape[0])]
    sol = [[] for _ in kbs]
    for j in range(assign.shape[0]):
        for b in assign[j]:
            sol[b] = [sizes[j]]
    return sizes, sol


def _order_slots(sizes):
    """Program order: sizes >= 2 ascending (small slots' loads land first and
    their exps cover the bigger slots' loads), 1-block slots last (their
    pacc output is P itself and PV is tiny, so the tail stays short)."""
    big = sorted(x for x in sizes if x >= 2)
    ones = [x for x in sizes if x == 1]
    return (big + ones) if big else ones


# ---------------------------------------------------------------- device

def _body(tc, qs, ks, vs, oTs, paccs, sizes):
    nc = tc.nc
    f32 = mybir.dt.float32
    f16 = mybir.dt.float16
    AF = mybir.ActivationFunctionType
    G = len(sizes)
    smax = max(sizes)

    with (
        tc.tile_pool(name="q", bufs=3) as q_pool,
        tc.tile_pool(name="k", bufs=3) as k_pool,
        tc.tile_pool(name="v", bufs=3) as v_pool,
        tc.tile_pool(name="p", bufs=2) as p_pool,
        tc.tile_pool(name="pa", bufs=2) as pa_pool,
        tc.tile_pool(name="eo", bufs=2) as e_pool,
        tc.tile_pool(name="const", bufs=1) as c_pool,
        tc.tile_pool(name="spsum", bufs=2, space="PSUM") as s_pool,
        tc.tile_pool(name="opsum", bufs=2, space="PSUM") as o_pool,
    ):
        def load_slot(g):
            s = sizes[g]
            sc = s * 128
            q_sb = q_pool.tile([128, Q], f16, tag="q", name=f"q{g}")
            k_sb = k_pool.tile([128, smax * 128], f16, tag="k", name=f"k{g}")
            v_sb = v_pool.tile([128, smax * 128], f16, tag="v", name=f"v{g}")
            if g <= 1:
                # the two rings transfer in parallel (~half the DMA bus
                # each), serial within a ring. Split each early slot's q
                # across both rings so S(g,blk0) unblocks as soon as
                # possible; k rides in front of the q halves (it is small
                # and LDWEIGHTS needs it first), v behind them.
                keng = nc.sync if g == 0 else nc.scalar
                nc.scalar.dma_start(out=q_sb[:, 0:QH], in_=qs[g][:, 0:QH])
                nc.sync.dma_start(out=q_sb[:, QH:Q], in_=qs[g][:, QH:Q])
                keng.dma_start(out=k_sb[:, 0:sc], in_=ks[g][:])
                (nc.sync if g == 0 else nc.scalar).dma_start(
                    out=v_sb[:, 0:sc], in_=vs[g][:])
            else:
                nc.sync.dma_start(out=k_sb[:, 0:sc], in_=ks[g][:])
                nc.sync.dma_start(out=q_sb[:], in_=qs[g][:])
                nc.sync.dma_start(out=v_sb[:, 0:sc], in_=vs[g][:])
            return q_sb, k_sb, v_sb

        def s_exp_one(g, blk, q_sb, k_sb, p_all):
            s_ps = s_pool.tile([128, Q], f32, tag="s", name=f"s{g}_{blk}")
            lhsT = k_sb[:, blk * 128 : (blk + 1) * 128]
            for qh in range(Q // QH):
                nc.tensor.matmul(
                    s_ps[:, qh * QH : (qh + 1) * QH],
                    lhsT,
                    q_sb[:, qh * QH : (qh + 1) * QH],
                    start=True,
                    stop=True,
                )
            nc.scalar.activation(
                p_all[:, blk * Q : (blk + 1) * Q], s_ps[:], AF.Exp, scale=SCALE
            )

        def s_exp_head(g, q_sb, k_sb):
            p_all = p_pool.tile([128, smax * Q], f16, tag="p", name=f"p{g}")
            for blk in range(min(2, sizes[g])):
                s_exp_one(g, blk, q_sb, k_sb, p_all)
            return p_all

        def s_exp_tail(g, q_sb, k_sb, p_all, start_blk):
            s = sizes[g]
            if s == 1:
                return None
            pacc = pa_pool.tile([128, Q], f16, tag="pa", name=f"pa{g}")
            nc.vector.tensor_add(pacc[:], p_all[:, 0:Q], p_all[:, Q : 2 * Q])
            for blk in range(2, s):
                if blk >= start_blk:
                    s_exp_one(g, blk, q_sb, k_sb, p_all)
                nc.vector.tensor_add(
                    pacc[:], pacc[:], p_all[:, blk * Q : (blk + 1) * Q]
                )
            return pacc

        def pv_out(g, p_all, v_sb, pacc, nxt=None):
            s = sizes[g]
            last = g == G - 1
            tailish = g >= G - 2
            # pacc output: P itself for 1-block pieces (no DVE work). The
            # last two slots' outputs ride the scalar ring (its exp stream
            # is over by then) so the two rings split the tail traffic.
            pacc_src = pacc[:] if pacc is not None else p_all[:, 0:Q]
            if tailish:
                nc.scalar.dma_start(out=paccs[g], in_=pacc_src)
            else:
                nc.sync.dma_start(out=paccs[g], in_=pacc_src)
            o_ps = [
                o_pool.tile([128, QH], f32, tag=f"o{qh}", name=f"o{g}_{qh}")
                for qh in range(Q // QH)
            ]
            for blk in range(s):
                for qh in range(Q // QH):
                    nc.tensor.matmul(
                        o_ps[qh][:],
                        v_sb[:, blk * 128 : (blk + 1) * 128],
                        p_all[:, blk * Q + qh * QH : blk * Q + (qh + 1) * QH],
                        start=(blk == 0),
                        stop=(blk == s - 1),
                    )
            # next slot's 3rd S tile + exp ride between PV and the evac so
            # the ScalarE stream stays dense across the slot boundary
            if nxt is not None:
                ng, nq, nk, np_all = nxt
                if sizes[ng] > 2:
                    s_exp_one(ng, 2, nq, nk, np_all)
            o_all = e_pool.tile([128, Q], f16, tag="oall", name=f"oall{g}")
            if last:
                # tail: halves split across DVE and ScalarE (exp stream done),
                # DMAs split across sync and scalar rings
                nc.vector.tensor_copy(o_all[:, 0:QH], o_ps[0][:])
                nc.sync.dma_start(out=oTs[g][:, 0:QH], in_=o_all[:, 0:QH])
                nc.scalar.copy(o_all[:, QH:Q], o_ps[1][:])
                nc.scalar.dma_start(out=oTs[g][:, QH:Q], in_=o_all[:, QH:Q])
            else:
                for qh in range(Q // QH):
                    nc.vector.tensor_copy(
                        o_all[:, qh * QH : (qh + 1) * QH], o_ps[qh][:]
                    )
                (nc.scalar if tailish else nc.sync).dma_start(
                    out=oTs[g], in_=o_all[:])

        # slot-0 loads first so their DMA issues are each ring's first work
        loads = [load_slot(0)]

        # dummy 1-column exp hoists the compiler-inserted ACT_TABLE_LOAD
        # (~1.3us) into the slot-0 DMA shadow
        scratch = c_pool.tile([128, 1], f16, tag="scratch", bufs=1)
        nc.gpsimd.memset(scratch[:], 1.0)
        nc.scalar.activation(scratch[:], scratch[:], AF.Exp, scale=1.0)

        # dummy matmuls ramp the PE p-state while slot-0 loads are in flight
        warm_w = c_pool.tile([128, QH], f16, tag="warmw", bufs=1)
        nc.gpsimd.memset(warm_w[:], 0.0)
        for w in range(N_WARM):
            warm_ps = s_pool.tile([128, QH], f32, tag="s", name=f"warm{w}")
            nc.tensor.matmul(warm_ps[:], warm_w[:, 0:128], warm_w[:],
                             start=True, stop=True)

        p_alls = [s_exp_head(0, loads[0][0], loads[0][1])]
        for g in range(G):
            if g + 1 < G:
                loads.append(load_slot(g + 1))
            q_sb, k_sb, v_sb = loads[g]
            pacc = s_exp_tail(g, q_sb, k_sb, p_alls[g],
                              start_blk=2 if g == 0 else 3)
            nxt = None
            if g + 1 < G:
                lq, lk, lv = loads[g + 1]
                p_alls.append(s_exp_head(g + 1, lq, lk))
                nxt = (g + 1, lq, lk, p_alls[g + 1])
            pv_out(g, p_alls[g], v_sb, pacc, nxt)


def _build(sizes):
    key = tuple(sizes)
    if key in _NC_CACHE:
        return _NC_CACHE[key]
    nc = bacc.Bacc("TRN2", target_bir_lowering=False, debug=False,
                   enable_asserts=False, enable_partition_id=False)
    f16 = mybir.dt.float16
    qs, ks, vs, oTs, paccs = [], [], [], [], []
    for g, s in enumerate(sizes):
        sc = s * 128
        qs.append(nc.dram_tensor(f"q{g}", [D, Q], f16,
                                 kind="ExternalInput").ap())
        ks.append(nc.dram_tensor(f"k{g}", [D, sc], f16,
                                 kind="ExternalInput").ap())
        vs.append(nc.dram_tensor(f"v{g}", [128, sc], f16,
                                 kind="ExternalInput").ap())
        oTs.append(nc.dram_tensor(f"oT{g}", [D, Q], f16,
                                  kind="ExternalOutput").ap())
        paccs.append(nc.dram_tensor(f"pacc{g}", [128, Q], f16,
                                    kind="ExternalOutput").ap())
    with tile.TileContext(nc) as tc:
        _body(tc, qs, ks, vs, oTs, paccs, sizes)
    nc.compile()
    _NC_CACHE[key] = nc
    return nc


# ---------------------------------------------------------------- host

def _prep(queries, keys, values, valid_length):
    """Returns (in_maps, pieces_by_cell, sizes).
    pieces_by_cell[(core, slot)] = (batch, k0_block, n_blocks) or None."""
    vl = np.asarray(valid_length).astype(np.int64).reshape(B)
    sizes_ms, per_batch = _find_packing(vl)
    sizes = _order_slots(sizes_ms)
    G = len(sizes)

    # cut each batch into contiguous pieces (largest piece first at k0=0)
    pieces_by_size: dict[int, list] = {}
    for b in range(B):
        k0 = 0
        for s in sorted(per_batch[b], reverse=True):
            pieces_by_size.setdefault(s, []).append((b, k0, s))
            k0 += s
    # fill cells slot by slot
    cells = {}
    for g in range(G):
        s = sizes[g]
        for c in range(N_CORES):
            lst = pieces_by_size.get(s, [])
            cells[(c, g)] = lst.pop() if lst else None
    assert all(not v for v in pieces_by_size.values()), "unassigned pieces"

    q = np.asarray(queries, dtype=np.float32)
    k = np.asarray(keys, dtype=np.float32)
    v = np.asarray(values, dtype=np.float32)
    qT_all = np.ascontiguousarray(q.transpose(0, 2, 1)).astype(np.float16)
    pos = np.arange(K)

    in_maps = []
    for c in range(N_CORES):
        m = {}
        for g in range(G):
            s = sizes[g]
            sc = s * 128
            piece = cells[(c, g)]
            if piece is None:
                m[f"q{g}"] = np.zeros((D, Q), np.float16)
                m[f"k{g}"] = np.zeros((D, sc), np.float16)
                m[f"v{g}"] = np.zeros((128, sc), np.float16)
                continue
            bi, k0, s_ = piece
            lo, hi = k0 * 128, k0 * 128 + sc
            valid = (pos[lo:hi] < vl[bi])[None, :]
            m[f"q{g}"] = qT_all[bi]
            m[f"k{g}"] = np.where(
                valid, k[bi, lo:hi].T, np.float32(0.0)
            ).astype(np.float16)
            vz = np.where(valid.T, v[bi, lo:hi], np.float32(0.0))
            m[f"v{g}"] = np.ascontiguousarray(
                vz.reshape(s, 128, D).transpose(1, 0, 2).reshape(128, sc)
            ).astype(np.float16)
        in_maps.append(m)
    return in_maps, cells, sizes, vl


def kernel(queries, keys, values, valid_length):
    global LAST_RESULTS
    in_maps, cells, sizes, vl = _prep(queries, keys, values, valid_length)
    nc = _build(sizes)
    res = run_bass_kernel_spmd(nc, in_maps, list(range(N_CORES)))
    LAST_RESULTS = res
    num = np.zeros((B, Q, D), np.float32)
    den = np.zeros((B, Q), np.float32)
    for c in range(N_CORES):
        rc = res.results[c]
        for g in range(len(sizes)):
            piece = cells[(c, g)]
            if piece is None:
                continue
            bi, k0, s = piece
            oT = np.asarray(rc[f"oT{g}"]).astype(np.float32)      # [D, Q]
            pacc = np.asarray(rc[f"pacc{g}"]).astype(np.float32)  # [128, Q]
            n_valid = int(np.clip(vl[bi] - 128 * k0, 0, 128 * s))
            n_masked = 128 * s - n_valid
            num[bi] += oT.T
            den[bi] += pacc.sum(axis=0) - np.float32(n_masked)
    return num / den[:, :, None]
